# revision 1
# baseline (speedup 1.0000x reference)
"""CIF (Continuous Integrate-and-Fire) forced-alignment kernel for 8 TRN2 NeuronCores.

Contract: kernel(**inputs) takes the FULL inputs from setup_inputs() and returns
the FULL (32, 8, 768) f32 output. Batch is sharded 4 seqs/core (pure data
parallel); each core runs an identical Bass/Tile program.

Math: Conv1d(D,D,5,SAME) + Linear(D,1) collapse into an effective (5,D) filter
(both are linear); sigmoid -> mask -> normalize to sum L -> cumsum -> bucket
overlap weights w (B,L,T) -> out = w @ x.

Device layout: t = p*J + j (p = partition, J = T/128). Projections via PE
transpose-mode matmuls (bf16), logits via padded shifted adds, cumsum via
free-dim prefix scan + strict-triangular matmul for cross-partition offsets,
einsum via PE matmuls with w stationary.
"""
import sys

if "/opt/trn_rl_repo" not in sys.path:
    sys.path.insert(0, "/opt/trn_rl_repo")

import numpy as np
import ml_dtypes

P = 128
BSZ, T_FULL, D_FULL = 32, 2048, 768
L_OUT = 8
N_CORES = 8
S_PER_CORE = BSZ // N_CORES


def fold_weights(conv_w, conv_b, lin_w, lin_b):
    Weff = np.einsum("o,oik->ki", lin_w[0].astype(np.float64),
                     conv_w.astype(np.float64)).astype(np.float32)  # (5, D)
    beff = float(lin_w[0].astype(np.float64) @ conv_b.astype(np.float64) + lin_b[0])
    return Weff, beff


def make_consts(S, T, D, L):
    J = T // P
    bf = ml_dtypes.bfloat16
    ident = np.eye(P, dtype=bf)
    i5 = np.eye(5, dtype=bf)
    u128x = np.triu(np.ones((P, P), np.float32), 1)         # [p,m]=1 iff p<m
    onescol = np.ones((P, 1), np.float32)
    onesrow = np.ones((1, P), np.float32)
    sup = np.zeros((P, P), bf)                              # out[m]=in[m+1]
    sup[np.arange(1, P), np.arange(P - 1)] = 1
    sdn = np.zeros((P, P), bf)                              # out[m]=in[m-1]
    sdn[np.arange(P - 1), np.arange(1, P)] = 1
    l = np.arange(L, dtype=np.float32)
    eglo = np.broadcast_to(l[None, :, None], (P, L, J)).copy()
    eghi = eglo + 1.0
    return dict(ident=ident, i5=i5, u128x=u128x, onescol=onescol,
                onesrow=onesrow, sup=sup, sdn=sdn, eglo=eglo, eghi=eghi)


def make_core_inputs(x_shard, lens_shard, Weff, beff, S, T, D, L):
    J = T // P
    CH = D // P
    bf = ml_dtypes.bfloat16
    wT = np.ascontiguousarray(
        Weff.reshape(5, CH, P).transpose(2, 1, 0)).astype(bf)       # (128, CH, 5)
    t_idx = (np.arange(P)[:, None] * J + np.arange(J)[None, :])     # (128, J)
    mb = np.where(t_idx[:, None, :] < np.asarray(lens_shard)[None, :, None],
                  np.float32(beff), np.float32(-30000.0)).astype(np.float32)
    m = dict(x=np.ascontiguousarray(x_shard, np.float32), wT=wT, mb=mb)
    m.update(make_consts(S, T, D, L))
    return m


def build_kernel(nc, tc, S, T, D, L, repeats=1):
    from concourse import mybir
    f32, bf16 = mybir.dt.float32, mybir.dt.bfloat16
    AF = mybir.ActivationFunctionType
    OP = mybir.AluOpType
    J = T // P
    CH = D // P
    JG = J // 4   # j-groups of 4

    x_d = nc.declare_dram_parameter("x", [S, T, D], f32, isOutput=False)
    wT_d = nc.declare_dram_parameter("wT", [P, CH, 5], bf16, isOutput=False)
    mb_d = nc.declare_dram_parameter("mb", [P, S, J], f32, isOutput=False)
    id_d = nc.declare_dram_parameter("ident", [P, P], bf16, isOutput=False)
    i5_d = nc.declare_dram_parameter("i5", [5, 5], bf16, isOutput=False)
    ux_d = nc.declare_dram_parameter("u128x", [P, P], f32, isOutput=False)
    oc_d = nc.declare_dram_parameter("onescol", [P, 1], f32, isOutput=False)
    or_d = nc.declare_dram_parameter("onesrow", [1, P], f32, isOutput=False)
    sup_d = nc.declare_dram_parameter("sup", [P, P], bf16, isOutput=False)
    sdn_d = nc.declare_dram_parameter("sdn", [P, P], bf16, isOutput=False)
    eglo_d = nc.declare_dram_parameter("eglo", [P, L, J], f32, isOutput=False)
    eghi_d = nc.declare_dram_parameter("eghi", [P, L, J], f32, isOutput=False)
    out_d = nc.declare_dram_parameter("out", [S, L, D], f32, isOutput=True)

    cpool = tc.alloc_tile_pool(name="consts", bufs=1)

    def load_const(dram, shape, dtype, cname):
        t = cpool.tile(shape, dtype, tag=cname, name=cname)
        nc.sync.dma_start(t[:], dram[:])
        return t

    ident = load_const(id_d, [P, P], bf16, "c_ident")
    i5 = load_const(i5_d, [5, 5], bf16, "c_i5")
    u128x = load_const(ux_d, [P, P], f32, "c_u128x")
    onescol = load_const(oc_d, [P, 1], f32, "c_onescol")
    onesrow = load_const(or_d, [1, P], f32, "c_onesrow")
    sup = load_const(sup_d, [P, P], bf16, "c_sup")
    sdn = load_const(sdn_d, [P, P], bf16, "c_sdn")
    eglo = load_const(eglo_d, [P, L, J], f32, "c_eglo")
    eghi = load_const(eghi_d, [P, L, J], f32, "c_eghi")
    wT = load_const(wT_d, [P, CH, 5], bf16, "c_wT")
    mb = load_const(mb_d, [P, S, J], f32, "c_mb")

    xbf_pool = tc.alloc_tile_pool(name="xbf", bufs=1)
    xbfs = [xbf_pool.tile([P, J, D], bf16, tag=f"xbf{s}", name=f"xbf{s}")
            for s in range(S)]

    xf_pool = tc.alloc_tile_pool(name="xf", bufs=2)
    xt_pool = tc.alloc_tile_pool(name="xt", bufs=2)
    z_pool = tc.alloc_tile_pool(name="zsb", bufs=1)
    zt_pool = tc.alloc_tile_pool(name="ztsb", bufs=1)
    sm_pool = tc.alloc_tile_pool(name="smsb", bufs=2)
    w_pool = tc.alloc_tile_pool(name="wsb", bufs=2)
    o_pool = tc.alloc_tile_pool(name="osb", bufs=2)

    tp_ps = tc.alloc_tile_pool(name="tp_ps", bufs=2, space="PSUM")
    z_ps_pool = tc.alloc_tile_pool(name="z_ps", bufs=2, space="PSUM")
    e_ps_pool = tc.alloc_tile_pool(name="e_ps", bufs=1, space="PSUM")
    s_ps_pool = tc.alloc_tile_pool(name="s_ps", bufs=2, space="PSUM")

    zsbs = [z_pool.tile([5, J, P], bf16, tag=f"z{s}", name=f"z{s}")
            for s in range(S)]
    ztps = [zt_pool.tile([P, J + 4, 5], bf16, tag=f"zt{s}", name=f"zt{s}")
            for s in range(S)]

    def body():
        for s in range(S):
            xbf = xbfs[s]
            zsb = zsbs[s]
            ztp = ztps[s]
            xsrc = x_d[s].rearrange("(p j) d -> p j d", j=J)
            for g in range(JG):
                xf = xf_pool.tile([P, 4, D], f32, tag="xf", name="xf")
                nc.sync.dma_start(xf[:], xsrc[:, 4 * g:4 * g + 4, :])
                conv_out = xbf[:, 4 * g:4 * g + 4, :]
                if g % 2 == 0:
                    nc.vector.tensor_copy(conv_out, xf[:])
                else:
                    nc.scalar.copy(conv_out, xf[:])
                z_ps = z_ps_pool.tile([5, 4 * P], f32, tag="zps", name="zps")
                for c in range(CH):
                    tp = tp_ps.tile([P, 4 * P], bf16, tag="tp", name="tp")
                    for q in range(4):
                        nc.tensor.transpose(
                            tp[:, q * P:(q + 1) * P],
                            xbf[:, 4 * g + q, c * P:(c + 1) * P],
                            ident[:],
                        )
                    xt = xt_pool.tile([P, 4 * P], bf16, tag="xt", name="xt")
                    nc.vector.tensor_copy(xt[:], tp[:])
                    nc.tensor.matmul(z_ps[:], wT[:, c, :], xt[:],
                                     start=(c == 0), stop=(c == CH - 1))
                nc.vector.tensor_copy(
                    zsb[:, 4 * g:4 * g + 4, :].rearrange("k a p -> k (a p)"),
                    z_ps[:])
            for b in range(JG):
                zt_psl = s_ps_pool.tile([P, 20], f32, tag="smps", name="ztps")
                for q in range(4):
                    j = 4 * b + q
                    nc.tensor.matmul(zt_psl[:, q * 5:(q + 1) * 5],
                                     zsb[:, j, :], i5[:], start=True, stop=True)
                nc.vector.tensor_copy(
                    ztp[:, 2 + 4 * b:2 + 4 * b + 4, :].rearrange("p a k -> p (a k)"),
                    zt_psl[:])
            fill = s_ps_pool.tile([P, 10], f32, tag="smps", name="fill")
            nc.tensor.matmul(fill[:], sup[:],
                             ztp[:, 2:4, :].rearrange("p a k -> p (a k)"),
                             start=True, stop=True)
            nc.vector.tensor_copy(
                ztp[:, J + 2:J + 4, :].rearrange("p a k -> p (a k)"), fill[:])
            fill2 = s_ps_pool.tile([P, 10], f32, tag="smps", name="fill")
            nc.tensor.matmul(fill2[:], sdn[:],
                             ztp[:, J:J + 2, :].rearrange("p a k -> p (a k)"),
                             start=True, stop=True)
            nc.vector.tensor_copy(
                ztp[:, 0:2, :].rearrange("p a k -> p (a k)"), fill2[:])
            # logits[p,j] = sum_k ztp[p, j+k, k]
            lg1 = sm_pool.tile([P, J], f32, tag="lg1", name="lg1")
            lg2 = sm_pool.tile([P, J], f32, tag="lg2", name="lg2")
            lg = sm_pool.tile([P, J], f32, tag="lg", name="lg")
            nc.vector.tensor_add(lg1[:], ztp[:, 2:2 + J, 2], ztp[:, 3:3 + J, 3])
            nc.vector.tensor_add(lg2[:], ztp[:, 4:4 + J, 4], ztp[:, 1:1 + J, 1])
            nc.vector.tensor_add(lg1[:], lg1[:], lg2[:])
            nc.vector.tensor_add(lg[:], lg1[:], ztp[:, 0:J, 0])
            nc.vector.tensor_add(lg[:], lg[:], mb[:, s, :])
            al = sm_pool.tile([P, J], f32, tag="al", name="al")
            tot = sm_pool.tile([P, 1], f32, tag="tot", name="tot")
            nc.scalar.activation(al[:], lg[:], AF.Sigmoid, accum_out=tot[:])
            A0 = sm_pool.tile([P, J], f32, tag="A0", name="A0")
            nc.vector.tensor_tensor_scan(A0[:], al[:], al[:], 0.0,
                                         op0=OP.add, op1=OP.bypass)
            offs = s_ps_pool.tile([P, 1], f32, tag="smps", name="offs")
            nc.tensor.matmul(offs[:], u128x[:], tot[:], start=True, stop=True)
            total = s_ps_pool.tile([1, 1], f32, tag="smps", name="total")
            nc.tensor.matmul(total[:], onescol[:], tot[:], start=True, stop=True)
            rec = sm_pool.tile([1, 1], f32, tag="rec", name="rec")
            nc.vector.reciprocal(rec[:], total[:])
            nc.scalar.mul(rec[:], rec[:], float(L))
            sbc = s_ps_pool.tile([P, 1], f32, tag="smps", name="sbc")
            nc.tensor.matmul(sbc[:], onesrow[:], rec[:], start=True, stop=True)
            sbv = sm_pool.tile([P, 1], f32, tag="sbv", name="sbv")
            nc.vector.tensor_copy(sbv[:], sbc[:])
            A1 = sm_pool.tile([P, J], f32, tag="A1", name="A1")
            nc.vector.tensor_add(A1[:], A0[:], offs[:].broadcast_to([P, J]))
            An = sm_pool.tile([P, J], f32, tag="An", name="An")
            nc.vector.tensor_mul(An[:], A1[:], sbv[:].broadcast_to([P, J]))
            aln = sm_pool.tile([P, J], f32, tag="aln", name="aln")
            nc.vector.tensor_mul(aln[:], al[:], sbv[:].broadcast_to([P, J]))
            Ap = sm_pool.tile([P, J], f32, tag="Ap", name="Ap")
            nc.vector.tensor_sub(Ap[:], An[:], aln[:])
            lo = w_pool.tile([P, L, J], f32, tag="lo", name="lo")
            nc.vector.tensor_max(
                lo[:],
                Ap[:].rearrange("p (o j) -> p o j", o=1).broadcast_to([P, L, J]),
                eglo[:])
            hi = w_pool.tile([P, L, J], f32, tag="hi", name="hi")
            nc.vector.tensor_tensor(
                hi[:],
                An[:].rearrange("p (o j) -> p o j", o=1).broadcast_to([P, L, J]),
                eghi[:], op=OP.min)
            wd = w_pool.tile([P, L, J], f32, tag="wd", name="wd")
            nc.vector.tensor_sub(wd[:], hi[:], lo[:])
            wbf = w_pool.tile([P, L, J], bf16, tag="wbf", name="wbf")
            nc.vector.tensor_scalar_max(wbf[:], wd[:], 0.0)
            e_ps = e_ps_pool.tile([L, 2, 512], f32, tag="eps", name="eps")
            for j in range(J):
                for h in range(2):
                    nc.tensor.matmul(e_ps[:, h, 0:D // 2], wbf[:, :, j],
                                     xbf[:, j, h * (D // 2):(h + 1) * (D // 2)],
                                     start=(j == 0), stop=(j == J - 1))
            osb = o_pool.tile([L, D], f32, tag="osb", name="osb")
            nc.vector.tensor_copy(osb[:, 0:D // 2], e_ps[:, 0, 0:D // 2])
            nc.vector.tensor_copy(osb[:, D // 2:D], e_ps[:, 1, 0:D // 2])
            nc.sync.dma_start(out_d[s], osb[:])

    if repeats == 1:
        body()
    else:
        with tc.For_i(0, repeats, 1):
            body()
    for pool in [s_ps_pool, e_ps_pool, z_ps_pool, tp_ps, o_pool, w_pool,
                 sm_pool, zt_pool, z_pool, xt_pool, xf_pool, xbf_pool, cpool]:
        pool.release()
    return nc


# ---------------------------------------------------------------------------
# Runner (persistent jitted SPMD dispatch via PJRT under axon)
# ---------------------------------------------------------------------------

_CACHE = {}


def _get_runner(repeats=1):
    key = ("runner", repeats)
    if key in _CACHE:
        return _CACHE[key]
    import concourse.tile as tile
    from concourse import bacc

    nc = bacc.Bacc()
    with tile.TileContext(nc) as tc:
        build_kernel(nc, tc, S_PER_CORE, T_FULL, D_FULL, L_OUT, repeats=repeats)
    nc.compile()
    runner = _SpmdRunner(nc, N_CORES)
    _CACHE[key] = runner
    return runner


class _SpmdRunner:
    def __init__(self, nc, n_cores):
        import jax
        import concourse.mybir as mybir
        from concourse.bass2jax import (_bass_exec_p, partition_id_tensor,
                                        install_neuronx_cc_hook)
        from jax.sharding import Mesh, PartitionSpec
        from jax.experimental.shard_map import shard_map

        install_neuronx_cc_hook()
        self.jax = jax
        self.nc = nc
        self.n_cores = n_cores
        partition_name = (nc.partition_id_tensor.name
                          if nc.partition_id_tensor else None)
        in_names, out_names, out_avals, zero_outs = [], [], [], []
        for alloc in nc.m.functions[0].allocations:
            if not isinstance(alloc, mybir.MemoryLocationSet):
                continue
            name = alloc.memorylocations[0].name
            if alloc.kind == "ExternalInput":
                if name != partition_name:
                    in_names.append(name)
            elif alloc.kind == "ExternalOutput":
                out_names.append(name)
                shape = tuple(alloc.tensor_shape)
                dtype = mybir.dt.np(alloc.dtype)
                out_avals.append(jax.core.ShapedArray(shape, dtype))
                zero_outs.append(np.zeros(shape, dtype))
        self.in_names, self.out_names = in_names, out_names
        self.out_avals, self.zero_outs = out_avals, zero_outs
        n_params = len(in_names)
        self.n_params = n_params
        all_in_names = list(in_names) + list(out_names)
        if partition_name is not None:
            all_in_names.append(partition_name)

        def _body(*args):
            operands = list(args)
            if partition_name is not None:
                operands.append(partition_id_tensor())
            outs = _bass_exec_p.bind(
                *operands,
                out_avals=tuple(out_avals),
                in_names=tuple(all_in_names),
                out_names=tuple(out_names),
                lowering_input_output_aliases=(),
                sim_require_finite=True,
                sim_require_nnan=True,
                nc=nc,
            )
            return tuple(outs)

        devices = jax.devices()[:n_cores]
        self.mesh = Mesh(np.asarray(devices), ("core",))
        n_outs = len(out_names)
        in_specs = (PartitionSpec("core"),) * (n_params + n_outs)
        out_specs = (PartitionSpec("core"),) * n_outs
        self.fn = jax.jit(
            shard_map(_body, mesh=self.mesh, in_specs=in_specs,
                      out_specs=out_specs, check_rep=False),
            keep_unused=True,
        )
        self._psharding = jax.sharding.NamedSharding(self.mesh,
                                                     PartitionSpec("core"))

    def device_inputs(self, in_maps):
        jax = self.jax
        per_core = [[np.asarray(m[n]) for n in self.in_names] for m in in_maps]
        concat_in = [
            np.concatenate([per_core[c][i] for c in range(self.n_cores)], axis=0)
            for i in range(self.n_params)
        ]
        concat_zeros = [
            np.zeros((self.n_cores * z.shape[0], *z.shape[1:]), z.dtype)
            for z in self.zero_outs
        ]
        return [jax.device_put(a, self._psharding)
                for a in concat_in + concat_zeros]

    def run(self, in_maps):
        jax = self.jax
        dev_in = self.device_inputs(in_maps)
        outs = self.fn(*dev_in)
        jax.block_until_ready(outs)
        return [
            {n: np.asarray(outs[i]).reshape(self.n_cores,
                                            *self.out_avals[i].shape)[c]
             for i, n in enumerate(self.out_names)}
            for c in range(self.n_cores)
        ]


def _make_in_maps(encoder_outputs, encoder_lens, conv_w, conv_b, lin_w, lin_b):
    Weff, beff = fold_weights(conv_w, conv_b, lin_w, lin_b)
    in_maps = []
    for c in range(N_CORES):
        sl = slice(c * S_PER_CORE, (c + 1) * S_PER_CORE)
        in_maps.append(make_core_inputs(
            np.asarray(encoder_outputs[sl], np.float32),
            np.asarray(encoder_lens[sl]),
            Weff, beff, S_PER_CORE, T_FULL, D_FULL, L_OUT))
    return in_maps


def kernel(encoder_outputs, encoder_lens, conv_w, conv_b, lin_w, lin_b):
    runner = _get_runner(repeats=1)
    in_maps = _make_in_maps(encoder_outputs, encoder_lens,
                            conv_w, conv_b, lin_w, lin_b)
    res = runner.run(in_maps)
    return np.concatenate([res[c]["out"] for c in range(N_CORES)], axis=0)
